# revision 18
# baseline (speedup 1.0000x reference)
"""Trainium2 Bass kernel for nn_Attention_21895743275585.

Reference computation (per batch b of 4):
  qkv = w_qkv @ x_flat            # 1x1 conv, x_flat [C=256, N=2304]
  q,k l2-normalized per (head, n) along dim_head=64; SCALE=10
  sim = 10 * qhat^T khat per head; attn = softmax(sim, axis=-1)
  out = attn @ v; final = w_out @ out_inner + b_out

Sharding: 8 cores = (batch b, head-half). Each core handles 4 of the 8 heads
of one batch; host sums the partial output projections (2 halves x 2 head
pairs per batch; bias is fed only to half 0 / pair 0).

The kernel is ACT(scalar engine)-bound: exp of the [N, N] sim matrices is
4 heads x 2304^2 = 21.2M elements at 1 elem/cycle/lane @1.2GHz ~= 160us.
Everything else is scheduled to hide under the exp stream:
  - attention runs chunk-major; each (chunk, j) group is one [128, 1024]
    PSUM->SBUF exp ACTIVATE (996ns steady cadence).
  - all other work (remaining QKV projection, v^T, norms, output projection,
    1/s scaling) is issued as small "side thunks" between attention groups so
    the PE/DVE/DMA work lands in the ACT-bound slack.
  - per-row math (1/sqrt(ss), 1/s) is repacked into dense [128, m] tiles via
    SBUF->SBUF/DRAM reshape DMAs so ACT/DVE process 128 partitions instead
    of 2-4 rows ([1,N]->[64,N] partition broadcasts bounce through small
    DRAM tensors -- DRAM APs allow a step-0 partition dim).
  - sim^T chunk [j, i] = k^T q in PSUM (two heads row-packed via
    tile_position); E@v packs two heads column-wise; denominators come from
    ones-column matmuls; exp needs no max subtraction since |sim|<=10.
  - Ln and Exp share one ACT table set (pinned natural_log_exp_and_others);
    rsqrt is exp(-0.5 ln x); 1/s is DVE reciprocal on packed tiles.
"""

import math
from collections import deque

import numpy as np

B, C, H, W = 4, 256, 48, 48
HEADS, DIM_HEAD, SCALE = 8, 64, 10.0
INNER = HEADS * DIM_HEAD
N = H * W                      # 2304
NJ = N // 128                  # 18 j-tiles
CHUNKS = [(0, 512), (512, 512), (1024, 512), (1536, 512), (2048, 256)]
EPS = 1e-12

WD_NAME = "bf16"               # working dtype: "bf16" | "f32r" | "f32"

_CACHE = {}


def _pin_act_tables():
    """Force every activation onto the natural_log_exp_and_others set so the
    whole kernel needs exactly one ACT table load (Ln+Exp share that set)."""
    import concourse.bacc as bacc_mod
    if getattr(bacc_mod, "_act_tables_pinned", False):
        return
    orig = bacc_mod.get_activation_tables

    def patched(arch):
        t = orig(arch)
        keep = "natural_log_exp_and_others"
        if keep in t:
            return {name: (funcs if name == keep else set())
                    for name, funcs in t.items()}
        return t

    bacc_mod.get_activation_tables = patched
    bacc_mod._act_tables_pinned = True


def _build(wd_name):
    import concourse.bass as bass
    import concourse.tile as tile
    from concourse import bacc, mybir

    _pin_act_tables()

    F32 = mybir.dt.float32
    F32R = mybir.dt.float32r
    WD = mybir.dt.bfloat16 if wd_name == "bf16" else F32

    def mc(ap):
        # matmul operand cast for the fast-fp32 PE path
        return ap.bitcast(F32R) if wd_name == "f32r" else ap

    Ln = mybir.ActivationFunctionType.Ln
    Exp = mybir.ActivationFunctionType.Exp
    ActCopy = mybir.ActivationFunctionType.Copy

    nc = bacc.Bacc("TRN2", target_bir_lowering=False, debug=False,
                   enable_asserts=False, num_devices=8)
    x2 = nc.dram_tensor("x2", [2, 128, N], WD, kind="ExternalInput").ap()
    wqk = nc.dram_tensor("wqk", [2, 128, 512], WD, kind="ExternalInput").ap()
    wvT = nc.dram_tensor("wvT", [2, 128, 256], WD, kind="ExternalInput").ap()
    woT = nc.dram_tensor("woT", [2, 128, 256], WD, kind="ExternalInput").ap()
    bias = nc.dram_tensor("bias", [2, 128, 1], F32, kind="ExternalInput").ap()
    ones8 = nc.dram_tensor("ones8", [128, 73], WD, kind="ExternalInput").ap()
    # output: per head-pair partial projections, summed on host
    y = nc.dram_tensor("y", [2, 2, 128, N], F32, kind="ExternalOutput").ap()
    # internal DRAM bounce rows for partition broadcasts (bf16: halves DMA
    # bytes and enables 2x DVE muls; ~0.2% rounding is within budget)
    # rsd rows: [q01(2), k01(2), q23(2), k23(2)] = 1/sqrt(ss) (q rows x10)
    rsd = nc.dram_tensor("rsd", [8, N], WD, kind="Internal").ap()
    rsdd = nc.dram_tensor("rsdd", [4, N], WD, kind="Internal").ap()

    with tile.TileContext(nc) as tc:
        with tc.tile_pool(name="persist", bufs=1) as P, \
             tc.tile_pool(name="bcast", bufs=1) as RSB, \
             tc.tile_pool(name="sq", bufs=3) as SQ, \
             tc.tile_pool(name="pk", bufs=2) as PK, \
             tc.tile_pool(name="esb", bufs=12) as ESB, \
             tc.tile_pool(name="yst", bufs=3) as YST, \
             tc.tile_pool(name="psf", bufs=2, space="PSUM") as PSF, \
             tc.tile_pool(name="pssim", bufs=2, space="PSUM") as PSSIM, \
             tc.tile_pool(name="pso", bufs=1, space="PSUM") as PSO:

            # ---- load inputs (wqk + x first: they gate the ramp) ----
            x_sb = [P.tile([128, N], WD, tag=f"x{c}", name=f"x{c}")
                    for c in range(2)]
            wqk_sb = [P.tile([128, 512], WD, tag=f"wqk{c}", name=f"wqk{c}")
                      for c in range(2)]
            wvT_sb = [P.tile([128, 256], WD, tag=f"wvT{c}", name=f"wvT{c}")
                      for c in range(2)]
            woT_sb = [P.tile([128, 256], WD, tag=f"woT{c}", name=f"woT{c}")
                      for c in range(2)]
            bias_sb = [P.tile([128, 1], F32, tag=f"bias{c}", name=f"bias{c}")
                       for c in range(2)]
            ones8_sb = P.tile([128, 73], WD, tag="ones8", name="ones8")
            for c in range(2):
                nc.sync.dma_start(wqk_sb[c][:, :], wqk[c])
                nc.sync.dma_start(x_sb[c][:, :], x2[c])
            nc.sync.dma_start(ones8_sb[:, :], ones8)
            for c in range(2):
                nc.sync.dma_start(wvT_sb[c][:, :], wvT[c])
                nc.sync.dma_start(woT_sb[c][:, :], woT[c])
                nc.sync.dma_start(bias_sb[c][:, :], bias[c])

            # all-partitions exp bias ln(SCALE) for q-row rsqrt packs
            biasq = P.tile([128, 1], F32, tag="biasq", name="biasq")
            nc.vector.memset(biasq[:, :], math.log(SCALE))

            qk_sb = [P.tile([128, N], WD, tag=f"qk{m}", name=f"qk{m}")
                     for m in range(4)]
            ss8 = P.tile([128, N], F32, tag="ss8", name="ss8")
            qhat = [P.tile([128, N], WD, tag=f"qh{p}", name=f"qh{p}")
                    for p in range(2)]
            khat = [P.tile([128, N], WD, tag=f"kh{p}", name=f"kh{p}")
                    for p in range(2)]
            vT_sb = P.tile([128, NJ, 4, 64], WD, tag="vT", name="vT")

            numer = [P.tile([128, N], WD, tag=f"nu{p}", name=f"nu{p}")
                     for p in range(2)]
            nsc = [P.tile([128, N], WD, tag=f"nsc{p}", name=f"nsc{p}")
                   for p in range(2)]
            s8 = P.tile([128, N], F32, tag="s8", name="s8")
            srec = P.tile([128, N], WD, tag="srec", name="srec")
            nc.vector.memset(s8[:, :], 1.0)

            # ---- building blocks ----

            # m tile channel order: 0=q01, 1=q23, 2=k01, 3=k23
            # ss8 row base for m: q01->0, k01->32, q23->64, k23->96
            SS_BASE = {0: 0, 2: 32, 1: 64, 3: 96}

            def qkv_chunk(m, off, cw, act_copy=False):
                """project chunk [off, off+cw) of qk tile m + its sum-sq."""
                base = SS_BASE[m]
                pq = PSF.tile([128, 512], F32, tag="pf", name="pq")
                for c in range(2):
                    nc.tensor.matmul(
                        pq[:, 0:cw],
                        mc(wqk_sb[c][:, m * 128:(m + 1) * 128]),
                        mc(x_sb[c][:, off:off + cw]),
                        start=(c == 0), stop=(c == 1))
                if act_copy:
                    nc.scalar.activation(qk_sb[m][:, off:off + cw],
                                         pq[:, 0:cw], ActCopy)
                else:
                    nc.vector.tensor_copy(qk_sb[m][:, off:off + cw],
                                          pq[:, 0:cw])
                q2 = SQ.tile([128, 512], WD, tag="q2", name="q2")
                nc.vector.tensor_mul(q2[:, 0:cw],
                                     qk_sb[m][:, off:off + cw],
                                     qk_sb[m][:, off:off + cw])
                pss = PSF.tile([8, 512], F32, tag="pf", name="pss")
                nc.tensor.matmul(pss[:, 0:cw], mc(ones8_sb[:, 0:8]),
                                 mc(q2[:, 0:cw]), start=True, stop=True)
                nc.vector.tensor_copy(ss8[base:base + 2, off:off + cw],
                                      pss[0:2, 0:cw])

            def rs_pack(ss_base, r0, off, cw, qscale):
                """packed 1/sqrt(ss): two ss8 rows [2, off:off+cw] ->
                [128, m] -> clamp, ln, exp(-0.5 ln + bias) -> rsd[r0:r0+2].
                q rows get bias=ln(SCALE) (rs = SCALE/||q||)."""
                mcols = (2 * cw) // 128
                pkin = PK.tile([128, 36], F32, tag="pk", name="pkin")
                pkrs = PK.tile([128, 36], WD, tag="pko", name="pkrs")
                nc.sync.dma_start(
                    pkin[:, 0:mcols],
                    ss8[ss_base:ss_base + 2, off:off + cw])
                nc.vector.tensor_scalar_max(pkin[:, 0:mcols],
                                            pkin[:, 0:mcols], EPS * EPS)
                nc.scalar.activation(pkin[:, 0:mcols], pkin[:, 0:mcols], Ln)
                if qscale:
                    nc.scalar.activation(pkrs[:, 0:mcols], pkin[:, 0:mcols],
                                         Exp, scale=-0.5, bias=biasq[:, :])
                else:
                    nc.scalar.activation(pkrs[:, 0:mcols], pkin[:, 0:mcols],
                                         Exp, scale=-0.5)
                nc.sync.dma_start(rsd[r0:r0 + 2, off:off + cw],
                                  pkrs[:, 0:mcols])

            # rsd row base: q01->0, k01->2, q23->4, k23->6
            def norm_part(p, which, off, cw, rsb):
                if which == "q":
                    dst, r0, src_m = qhat[p], 4 * p, p
                else:
                    dst, r0, src_m = khat[p], 4 * p + 2, 2 + p
                row = rsd[r0][off:off + cw]
                src = bass.AP(tensor=row.tensor, offset=row.offset,
                              ap=[[N, 2], [0, 64]] + list(row.ap))
                nc.sync.dma_start(rsb[:, off:off + cw], src)
                with nc.allow_low_precision(reason="bf16 norm mul"):
                    nc.vector.tensor_mul(dst[:, off:off + cw],
                                         qk_sb[src_m][:, off:off + cw],
                                         rsb[:, off:off + cw])

            def vt_tile(jt, pool=None, act_copy=False):
                pv = (pool or PSF).tile([128, 256], F32,
                                        tag="pf" if pool is None else "ps",
                                        name="pv")
                for c in range(2):
                    nc.tensor.matmul(
                        pv[:, :],
                        mc(x_sb[c][:, jt * 128:(jt + 1) * 128]),
                        mc(wvT_sb[c][:, :]),
                        start=(c == 0), stop=(c == 1))
                if act_copy:
                    nc.scalar.activation(
                        vT_sb[:, jt, :, :],
                        pv.rearrange("p (h d) -> p h d", h=4), ActCopy)
                else:
                    nc.vector.tensor_copy(
                        vT_sb[:, jt, :, :],
                        pv.rearrange("p (h d) -> p h d", h=4))

            def recip_chunk(hp, off, cw):
                """packed 1/s for pair hp chunk: s8 rows {64hp, 64hp+32}
                -> [128, m] -> DVE reciprocal -> rsdd[2hp:2hp+2].
                (reciprocal costs ~7 cyc/free-elem, so packing across
                partitions is essential)"""
                mcols = (2 * cw) // 128
                pks = PK.tile([128, 8], F32, tag="pks", name="pks")
                pkr = PK.tile([128, 8], WD, tag="pko", name="pkr")
                nc.sync.dma_start(pks[0:64, 0:mcols],
                                  s8[64 * hp:64 * hp + 1, off:off + cw])
                nc.sync.dma_start(pks[64:128, 0:mcols],
                                  s8[64 * hp + 32:64 * hp + 33,
                                     off:off + cw])
                with nc.allow_low_precision(reason="bf16 1/s"):
                    nc.vector.reciprocal(pkr[:, 0:mcols], pks[:, 0:mcols])
                nc.sync.dma_start(rsdd[2 * hp:2 * hp + 2, off:off + cw],
                                  pkr[:, 0:mcols])

            def nsc_chunk(hp, off, cw, rsb):
                row = rsdd[2 * hp][off:off + cw]
                src = bass.AP(tensor=row.tensor, offset=row.offset,
                              ap=[[N, 2], [0, 64]] + list(row.ap))
                nc.sync.dma_start(rsb[:, off:off + cw], src)
                with nc.allow_low_precision(reason="bf16 nsc mul"):
                    nc.vector.tensor_mul(nsc[hp][:, off:off + cw],
                                         numer[hp][:, off:off + cw],
                                         rsb[:, off:off + cw])

            def tail_scale(hp, off, cw):
                """latency-optimized last-chunk 1/s: row reciprocal (no DMA
                bounce) + K=1 ones-matmul partition broadcast."""
                b = 64 * hp
                with nc.allow_low_precision(reason="bf16 1/s"):
                    nc.vector.reciprocal(srec[b:b + 33, off:off + cw],
                                         s8[b:b + 33, off:off + cw])
                pb = PSF.tile([128, 512], F32, tag="pf", name="pb")
                nc.tensor.matmul(
                    pb[0:64, 0:cw],
                    mc(ones8_sb[b:b + 1, 9:73]),
                    mc(srec[b:b + 1, off:off + cw]),
                    start=True, stop=True, tile_position=(b, 0))
                nc.tensor.matmul(
                    pb[64:128, 0:cw],
                    mc(ones8_sb[b + 32:b + 33, 9:73]),
                    mc(srec[b + 32:b + 33, off:off + cw]),
                    start=True, stop=True, tile_position=(b + 32, 64))
                with nc.allow_low_precision(reason="bf16 nsc mul"):
                    nc.vector.tensor_mul(nsc[hp][:, off:off + cw],
                                         numer[hp][:, off:off + cw],
                                         pb[:, 0:cw])

            def outproj_piece(pr, m2, off, cw):
                pf = PSF.tile([128, 512], F32, tag="pf", name="pf")
                nc.tensor.matmul(
                    pf[:, 0:cw],
                    mc(woT_sb[pr][:, m2 * 128:(m2 + 1) * 128]),
                    mc(nsc[pr][:, off:off + cw]),
                    start=True, stop=True)
                yt = YST.tile([128, 512], F32, tag="yt", name="yt")
                if pr == 0:
                    nc.vector.tensor_scalar_add(yt[:, 0:cw], pf[:, 0:cw],
                                                bias_sb[m2][:, :])
                else:
                    nc.vector.tensor_copy(yt[:, 0:cw], pf[:, 0:cw])
                nc.sync.dma_start(y[pr][m2][:, off:off + cw], yt[:, 0:cw])

            # ---- attention core (ACT-bound steady state) ----
            # side: deque of thunks issued between j-groups (PE slack);
            # cadence: issue one thunk every `cadence` groups.
            def attention_pair(hp, off, cw, side=None, cadence=3):
                po = PSO.tile([128, 512], F32, tag="po", name="po")
                po_o = PSO.tile([33, 512], F32, tag="po_o", name="po_o")

                def sim_pair(jt, ps):
                    js = slice(jt * 128, (jt + 1) * 128)
                    nc.tensor.matmul(
                        ps[:, 0:cw],
                        mc(khat[hp][0:64, js]),
                        mc(qhat[hp][0:64, off:off + cw]),
                        start=True, stop=True, tile_position=(0, 0))
                    nc.tensor.matmul(
                        ps[:, 512:512 + cw],
                        mc(khat[hp][64:128, js]),
                        mc(qhat[hp][64:128, off:off + cw]),
                        start=True, stop=True, tile_position=(64, 0))

                def ev_group(jt, eh0, eh1):
                    st, sp = (jt == 0), (jt == NJ - 1)
                    nc.tensor.matmul(
                        po[0:64, 0:cw],
                        mc(vT_sb[:, jt, 2 * hp, :]),
                        mc(eh0),
                        start=st, stop=sp, tile_position=(0, 0),
                        skip_group_check=True)
                    nc.tensor.matmul(
                        po[64:128, 0:cw],
                        mc(vT_sb[:, jt, 2 * hp + 1, :]),
                        mc(eh1),
                        start=st, stop=sp, tile_position=(0, 64),
                        skip_group_check=True)
                    nc.tensor.matmul(
                        po_o[0:1, 0:cw],
                        mc(ones8_sb[:, 8:9]),
                        mc(eh0),
                        start=st, stop=sp, tile_position=(0, 0),
                        skip_group_check=True)
                    nc.tensor.matmul(
                        po_o[32:33, 0:cw],
                        mc(ones8_sb[:, 8:9]),
                        mc(eh1),
                        start=st, stop=sp, tile_position=(0, 32),
                        skip_group_check=True)

                # E@v trails 3 j's behind so the next chunk's first E@v
                # (which waits the previous chunk's po drain) never blocks
                # early sims on the in-order PE
                pend = []
                for jt in range(NJ):
                    ps = PSSIM.tile([128, 1024], F32, tag="ps", name="ps")
                    sim_pair(jt, ps)
                    e = ESB.tile([128, 1024], WD, tag="e", name="e")
                    ps3 = ps.rearrange("p (b c) -> p b c", b=2)
                    e3b = e.rearrange("p (b c) -> p b c", b=2)
                    nc.scalar.activation(e3b[:, :, 0:cw],
                                         ps3[:, :, 0:cw], Exp)
                    pend.append((jt, e))
                    if len(pend) > 3:
                        j0, ee = pend.pop(0)
                        ev_group(j0, ee[:, 0:cw], ee[:, 512:512 + cw])
                    if side and (jt % cadence == cadence - 1):
                        if len(side):
                            side.popleft()()
                for (j0, ee) in pend:
                    ev_group(j0, ee[:, 0:cw], ee[:, 512:512 + cw])
                # drain numerators + denominators (s rows at base 32h)
                nc.vector.tensor_copy(numer[hp][:, off:off + cw],
                                      po[:, 0:cw])
                for t in range(2):
                    h = 2 * hp + t
                    nc.vector.tensor_copy(
                        s8[32 * h:32 * h + 1, off:off + cw],
                        po_o[32 * t:32 * t + 1, 0:cw])

            # ---- schedule ----
            # PE warm-up: ~4us of dummy matmuls during the input-DMA wait
            # flips the HAM clock gate to 2.4GHz before real work arrives.
            wtile = SQ.tile([128, 512], WD, tag="q2", name="wtile")
            nc.vector.memset(wtile[:, :], 0.0)
            for _ in range(10):
                pw = PSF.tile([128, 512], F32, tag="pf", name="pw")
                nc.tensor.matmul(pw[:, :], mc(wtile[:, 0:128]),
                                 mc(wtile[:, :]), start=True, stop=True)

            # ramp: k01 full row (chunk-pipelined rs), q01 chunk 0, v^T via
            # the idle PSSIM banks; sim j-tiles span all of khat so the k
            # row must be complete, qhat only needs chunk 0.
            rsbq0 = RSB.tile([128, N], WD, tag="rsbq0", name="rsbq0")
            rsbk0 = RSB.tile([128, N], WD, tag="rsbk0", name="rsbk0")
            rsbq1 = RSB.tile([128, N], WD, tag="rsbq1", name="rsbq1")
            rsbk1 = RSB.tile([128, N], WD, tag="rsbk1", name="rsbk1")

            off0, cw0 = CHUNKS[0]
            for (off, cw) in CHUNKS:
                qkv_chunk(2, off, cw)                   # k01
                rs_pack(32, 2, off, cw, qscale=False)
            qkv_chunk(0, off0, cw0)                     # q01 chunk 0
            rs_pack(0, 0, off0, cw0, qscale=True)
            for (off, cw) in CHUNKS:
                norm_part(0, "k", off, cw, rsbk0)
            norm_part(0, "q", off0, cw0, rsbq0)
            for jt in range(6):
                vt_tile(jt, pool=PSSIM)

            # side work queues; prep for pair-0 chunk c (qhat) must finish
            # inside chunks 0..c-1; pair-1's k row inside pair-0's window.
            side_lists = [deque() for _ in CHUNKS]

            def make_qkv_thunk(m, off, cw):
                return lambda: qkv_chunk(m, off, cw)

            def make_rs_thunk(base, r0, off, cw, qs):
                return lambda: rs_pack(base, r0, off, cw, qs)

            def make_norm_thunk(p, which, off, cw, rsb):
                return lambda: norm_part(p, which, off, cw, rsb)

            def add_q_prep(dst, p, ci):
                off, cw = CHUNKS[ci]
                m_q = 0 if p == 0 else 1
                rq = rsbq0 if p == 0 else rsbq1
                dst.append(make_qkv_thunk(m_q, off, cw))
                dst.append(make_rs_thunk(64 * p, 4 * p, off, cw, True))
                dst.append(make_norm_thunk(p, "q", off, cw, rq))

            def make_vt_thunk(jt):
                return lambda: vt_tile(jt)

            for jt in range(6, NJ):
                side_lists[0].append(make_vt_thunk(jt))
            add_q_prep(side_lists[0], 0, 1)
            add_q_prep(side_lists[1], 0, 2)
            add_q_prep(side_lists[1], 0, 3)
            add_q_prep(side_lists[2], 0, 4)
            # pair-1 k row: project k23 + rs + chunked norm before pair-1
            for ci, (off, cw) in enumerate(CHUNKS):
                dst = side_lists[2] if ci < 3 else side_lists[3]
                dst.append(make_qkv_thunk(3, off, cw))
                dst.append(make_rs_thunk(96, 6, off, cw, False))
            for (off, cw) in CHUNKS:
                side_lists[3].append(
                    make_norm_thunk(1, "k", off, cw, rsbk1))
            add_q_prep(side_lists[3], 1, 0)

            cadences0 = [1, 3, 2, 1, 2]
            for ci, (off, cw) in enumerate(CHUNKS):
                attention_pair(0, off, cw, side=side_lists[ci],
                               cadence=cadences0[ci])
                while side_lists[ci]:
                    side_lists[ci].popleft()()

            # pair-1 attention with pair-0 scaling/outproj, remaining pair-1
            # q prep, and trailing pair-1 scaling/outproj as side work
            side1 = [deque() for _ in CHUNKS]

            def make_recip_thunk(hp, off, cw):
                return lambda: recip_chunk(hp, off, cw)

            rsb0 = RSB.tile([128, N], WD, tag="rsb0", name="rsb0")
            rsb1 = RSB.tile([128, N], WD, tag="rsb1", name="rsb1")

            def make_nsc_thunk(hp, off, cw):
                rsb = rsb0 if hp == 0 else rsb1
                return lambda: nsc_chunk(hp, off, cw, rsb)

            def make_out_thunk(pr, m2, off, cw):
                return lambda: outproj_piece(pr, m2, off, cw)

            for ci in range(1, len(CHUNKS)):
                add_q_prep(side1[ci - 1], 1, ci)
            for ci, (off, cw) in enumerate(CHUNKS):
                side1[ci].append(make_recip_thunk(0, off, cw))
                side1[ci].append(make_nsc_thunk(0, off, cw))
                side1[ci].append(make_out_thunk(0, 0, off, cw))
                side1[ci].append(make_out_thunk(0, 1, off, cw))
                if ci >= 1:
                    poff, pcw = CHUNKS[ci - 1]
                    side1[ci].append(make_recip_thunk(1, poff, pcw))
                    side1[ci].append(make_nsc_thunk(1, poff, pcw))
                    side1[ci].append(make_out_thunk(1, 0, poff, pcw))
                    side1[ci].append(make_out_thunk(1, 1, poff, pcw))

            cadences1 = [2, 1, 1, 1, 2]
            for ci, (off, cw) in enumerate(CHUNKS):
                attention_pair(1, off, cw, side=side1[ci],
                               cadence=cadences1[ci])
                while side1[ci]:
                    side1[ci].popleft()()

            # tail: last chunk of pair-1 scaling + outproj (low-latency
            # path: no DRAM bounce)
            offl, cwl = CHUNKS[-1]
            tail_scale(1, offl, cwl)
            outproj_piece(1, 0, offl, cwl)
            outproj_piece(1, 1, offl, cwl)

    nc.compile()
    return nc


def _get_program(wd_name=WD_NAME):
    if wd_name not in _CACHE:
        _CACHE[wd_name] = _build(wd_name)
    return _CACHE[wd_name]


def _np_wd(wd_name):
    if wd_name == "bf16":
        import ml_dtypes
        return np.dtype(ml_dtypes.bfloat16)
    return np.dtype(np.float32)


def make_in_maps(x, w_qkv, w_out, b_out, wd_name=WD_NAME):
    x = np.asarray(x, np.float32)
    w_qkv = np.asarray(w_qkv, np.float32)
    w_out = np.asarray(w_out, np.float32)
    b_out = np.asarray(b_out, np.float32)
    wd = _np_wd(wd_name)

    ones8 = np.zeros((128, 73), np.float32)
    ones8[:, 8:] = 1.0
    for cc in range(8):
        lo = 64 * (cc % 2)
        ones8[lo:lo + 64, cc] = 1.0

    in_maps = []
    for core in range(8):
        b, half = core // 2, core % 2
        hsel = slice(256 * half, 256 * (half + 1))
        q_rows = np.arange(0, 512)[hsel]
        k_rows = 512 + q_rows
        v_rows = 1024 + q_rows
        wqk_h = np.ascontiguousarray(
            w_qkv[np.r_[q_rows, k_rows], :].T).reshape(2, 128, 512)
        wvT_h = np.ascontiguousarray(w_qkv[v_rows, :].T).reshape(2, 128, 256)
        woT_h = np.ascontiguousarray(w_out[:, hsel].T).reshape(2, 128, 256)
        bias_h = (b_out if half == 0 else np.zeros_like(b_out))
        in_maps.append({
            "x2": x[b].reshape(C, N).reshape(2, 128, N).astype(wd),
            "wqk": wqk_h.astype(wd),
            "wvT": wvT_h.astype(wd),
            "woT": woT_h.astype(wd),
            "bias": bias_h.reshape(2, 128, 1).astype(np.float32),
            "ones8": ones8.astype(wd),
        })
    return in_maps


def gather_output(results):
    outs = [r["y"].sum(axis=0).reshape(C, N) for r in results]
    return np.stack([
        (outs[2 * b] + outs[2 * b + 1]).reshape(C, H, W) for b in range(B)
    ]).astype(np.float32)


def run(in_maps, wd_name=WD_NAME, **kwargs):
    from concourse import bass_utils
    nc = _get_program(wd_name)
    return bass_utils.run_bass_kernel_spmd(nc, in_maps,
                                           core_ids=list(range(8)), **kwargs)


def kernel(x, w_qkv, w_out, b_out):
    in_maps = make_in_maps(x, w_qkv, w_out, b_out)
    res = run(in_maps)
    return gather_output(res.results)


# revision 19
# speedup vs baseline: 1.1324x; 1.1324x over previous
"""Trainium2 Bass kernel for nn_Attention_21895743275585.

Reference computation (per batch b of 4):
  qkv = w_qkv @ x_flat            # 1x1 conv, x_flat [C=256, N=2304]
  q,k l2-normalized per (head, n) along dim_head=64; SCALE=10
  sim = 10 * qhat^T khat per head; attn = softmax(sim, axis=-1)
  out = attn @ v; final = w_out @ out_inner + b_out

Sharding: 8 cores = (batch b, head-half). Each core handles 4 of the 8 heads
of one batch; host sums the partial output projections (2 halves x 2 head
pairs per batch; bias is fed only to half 0 / pair 0).

The kernel is ACT(scalar engine)-bound: exp of the [N, N] sim matrices is
4 heads x 2304^2 = 21.2M elements at 1 elem/cycle/lane @1.2GHz ~= 160us.
Everything else is scheduled to hide under the exp stream:
  - attention runs chunk-major; each (chunk, j) group is one [128, 1024]
    PSUM->SBUF exp ACTIVATE (996ns steady cadence).
  - all other work (remaining QKV projection, v^T, norms, output projection,
    1/s scaling) is issued as small "side thunks" between attention groups so
    the PE/DVE/DMA work lands in the ACT-bound slack.
  - per-row math (1/sqrt(ss), 1/s) is repacked into dense [128, m] tiles via
    SBUF->SBUF/DRAM reshape DMAs so ACT/DVE process 128 partitions instead
    of 2-4 rows ([1,N]->[64,N] partition broadcasts bounce through small
    DRAM tensors -- DRAM APs allow a step-0 partition dim).
  - sim^T chunk [j, i] = k^T q in PSUM (two heads row-packed via
    tile_position); E@v packs two heads column-wise; denominators come from
    ones-column matmuls; exp needs no max subtraction since |sim|<=10.
  - Ln and Exp share one ACT table set (pinned natural_log_exp_and_others);
    rsqrt is exp(-0.5 ln x); 1/s is DVE reciprocal on packed tiles.
"""

import math
from collections import deque

import numpy as np

B, C, H, W = 4, 256, 48, 48
HEADS, DIM_HEAD, SCALE = 8, 64, 10.0
INNER = HEADS * DIM_HEAD
N = H * W                      # 2304
NJ = N // 128                  # 18 j-tiles
CHUNKS = [(0, 512), (512, 512), (1024, 512), (1536, 512), (2048, 256)]
EPS = 1e-12

WD_NAME = "bf16"               # working dtype: "bf16" | "f32r" | "f32"

_CACHE = {}


def _pin_act_tables():
    """Force every activation onto the natural_log_exp_and_others set so the
    whole kernel needs exactly one ACT table load (Ln+Exp share that set)."""
    import concourse.bacc as bacc_mod
    if getattr(bacc_mod, "_act_tables_pinned", False):
        return
    orig = bacc_mod.get_activation_tables

    def patched(arch):
        t = orig(arch)
        keep = "natural_log_exp_and_others"
        if keep in t:
            return {name: (funcs if name == keep else set())
                    for name, funcs in t.items()}
        return t

    bacc_mod.get_activation_tables = patched
    bacc_mod._act_tables_pinned = True


def _build(wd_name):
    import concourse.bass as bass
    import concourse.tile as tile
    from concourse import bacc, mybir

    _pin_act_tables()

    F32 = mybir.dt.float32
    F32R = mybir.dt.float32r
    WD = mybir.dt.bfloat16 if wd_name == "bf16" else F32

    def mc(ap):
        # matmul operand cast for the fast-fp32 PE path
        return ap.bitcast(F32R) if wd_name == "f32r" else ap

    Ln = mybir.ActivationFunctionType.Ln
    Exp = mybir.ActivationFunctionType.Exp
    ActCopy = mybir.ActivationFunctionType.Copy

    nc = bacc.Bacc("TRN2", target_bir_lowering=False, debug=False,
                   enable_asserts=False, num_devices=8)
    x2 = nc.dram_tensor("x2", [2, 128, N], WD, kind="ExternalInput").ap()
    wqk = nc.dram_tensor("wqk", [2, 128, 512], WD, kind="ExternalInput").ap()
    wvT = nc.dram_tensor("wvT", [2, 128, 256], WD, kind="ExternalInput").ap()
    woT = nc.dram_tensor("woT", [2, 128, 256], WD, kind="ExternalInput").ap()
    bias = nc.dram_tensor("bias", [2, 128, 1], F32, kind="ExternalInput").ap()
    ones8 = nc.dram_tensor("ones8", [128, 73], WD, kind="ExternalInput").ap()
    # output: per head-pair partial projections, summed on host
    y = nc.dram_tensor("y", [2, 2, 128, N], F32, kind="ExternalOutput").ap()
    # internal DRAM bounce rows for partition broadcasts (bf16: halves DMA
    # bytes and enables 2x DVE muls; ~0.2% rounding is within budget)
    # rsd rows: [q01(2), k01(2), q23(2), k23(2)] = 1/sqrt(ss) (q rows x10)
    rsd = nc.dram_tensor("rsd", [8, N], WD, kind="Internal").ap()
    rsdd = nc.dram_tensor("rsdd", [4, N], WD, kind="Internal").ap()

    with tile.TileContext(nc) as tc:
        with tc.tile_pool(name="persist", bufs=1) as P, \
             tc.tile_pool(name="bcast", bufs=1) as RSB, \
             tc.tile_pool(name="sq", bufs=3) as SQ, \
             tc.tile_pool(name="pk", bufs=2) as PK, \
             tc.tile_pool(name="esb", bufs=12) as ESB, \
             tc.tile_pool(name="yst", bufs=3) as YST, \
             tc.tile_pool(name="psf", bufs=2, space="PSUM") as PSF, \
             tc.tile_pool(name="pssim", bufs=2, space="PSUM") as PSSIM, \
             tc.tile_pool(name="pso", bufs=1, space="PSUM") as PSO:

            # ---- load inputs (wqk + x first: they gate the ramp) ----
            x_sb = [P.tile([128, N], WD, tag=f"x{c}", name=f"x{c}")
                    for c in range(2)]
            wqk_sb = [P.tile([128, 512], WD, tag=f"wqk{c}", name=f"wqk{c}")
                      for c in range(2)]
            wvT_sb = [P.tile([128, 256], WD, tag=f"wvT{c}", name=f"wvT{c}")
                      for c in range(2)]
            woT_sb = [P.tile([128, 256], WD, tag=f"woT{c}", name=f"woT{c}")
                      for c in range(2)]
            bias_sb = [P.tile([128, 1], F32, tag=f"bias{c}", name=f"bias{c}")
                       for c in range(2)]
            ones8_sb = P.tile([128, 73], WD, tag="ones8", name="ones8")
            for c in range(2):
                nc.sync.dma_start(wqk_sb[c][:, :], wqk[c])
                nc.sync.dma_start(x_sb[c][:, :], x2[c])
            nc.sync.dma_start(ones8_sb[:, :], ones8)
            for c in range(2):
                nc.sync.dma_start(wvT_sb[c][:, :], wvT[c])
                nc.sync.dma_start(woT_sb[c][:, :], woT[c])
                nc.sync.dma_start(bias_sb[c][:, :], bias[c])

            # all-partitions exp bias ln(SCALE) for q-row rsqrt packs
            biasq = P.tile([128, 1], F32, tag="biasq", name="biasq")
            nc.vector.memset(biasq[:, :], math.log(SCALE))

            qk_sb = [P.tile([128, N], WD, tag=f"qk{m}", name=f"qk{m}")
                     for m in range(4)]
            ss8 = P.tile([128, N], F32, tag="ss8", name="ss8")
            qhat = [P.tile([128, N], WD, tag=f"qh{p}", name=f"qh{p}")
                    for p in range(2)]
            khat = [P.tile([128, N], WD, tag=f"kh{p}", name=f"kh{p}")
                    for p in range(2)]
            vT_sb = P.tile([128, NJ, 4, 64], WD, tag="vT", name="vT")

            numer = [P.tile([128, N], WD, tag=f"nu{p}", name=f"nu{p}")
                     for p in range(2)]
            nsc = [P.tile([128, N], WD, tag=f"nsc{p}", name=f"nsc{p}")
                   for p in range(2)]
            s8 = P.tile([128, N], F32, tag="s8", name="s8")
            srec = P.tile([128, N], WD, tag="srec", name="srec")
            nc.vector.memset(s8[:, :], 1.0)

            # ---- building blocks ----

            # m tile channel order: 0=q01, 1=q23, 2=k01, 3=k23
            # ss8 row base for m: q01->0, k01->32, q23->64, k23->96
            SS_BASE = {0: 0, 2: 32, 1: 64, 3: 96}

            def qkv_chunk(m, off, cw, act_copy=False):
                """project chunk [off, off+cw) of qk tile m + its sum-sq."""
                base = SS_BASE[m]
                pq = PSF.tile([128, 512], F32, tag="pf", name="pq")
                for c in range(2):
                    nc.tensor.matmul(
                        pq[:, 0:cw],
                        mc(wqk_sb[c][:, m * 128:(m + 1) * 128]),
                        mc(x_sb[c][:, off:off + cw]),
                        start=(c == 0), stop=(c == 1))
                if act_copy:
                    nc.scalar.activation(qk_sb[m][:, off:off + cw],
                                         pq[:, 0:cw], ActCopy)
                else:
                    nc.vector.tensor_copy(qk_sb[m][:, off:off + cw],
                                          pq[:, 0:cw])
                q2 = SQ.tile([128, 512], WD, tag="q2", name="q2")
                nc.vector.tensor_mul(q2[:, 0:cw],
                                     qk_sb[m][:, off:off + cw],
                                     qk_sb[m][:, off:off + cw])
                pss = PSF.tile([8, 512], F32, tag="pf", name="pss")
                nc.tensor.matmul(pss[:, 0:cw], mc(ones8_sb[:, 0:8]),
                                 mc(q2[:, 0:cw]), start=True, stop=True)
                if act_copy:
                    nc.scalar.activation(ss8[base:base + 2, off:off + cw],
                                         pss[0:2, 0:cw], ActCopy)
                else:
                    nc.vector.tensor_copy(ss8[base:base + 2, off:off + cw],
                                          pss[0:2, 0:cw])

            def rs_pack(ss_base, r0, off, cw, qscale):
                """packed 1/sqrt(ss): two ss8 rows [2, off:off+cw] ->
                [128, m] -> clamp, ln, exp(-0.5 ln + bias) -> rsd[r0:r0+2].
                q rows get bias=ln(SCALE) (rs = SCALE/||q||)."""
                mcols = (2 * cw) // 128
                pkin = PK.tile([128, 36], F32, tag="pk", name="pkin")
                pkrs = PK.tile([128, 36], WD, tag="pko", name="pkrs")
                nc.sync.dma_start(
                    pkin[:, 0:mcols],
                    ss8[ss_base:ss_base + 2, off:off + cw])
                nc.vector.tensor_scalar_max(pkin[:, 0:mcols],
                                            pkin[:, 0:mcols], EPS * EPS)
                nc.scalar.activation(pkin[:, 0:mcols], pkin[:, 0:mcols], Ln)
                if qscale:
                    nc.scalar.activation(pkrs[:, 0:mcols], pkin[:, 0:mcols],
                                         Exp, scale=-0.5, bias=biasq[:, :])
                else:
                    nc.scalar.activation(pkrs[:, 0:mcols], pkin[:, 0:mcols],
                                         Exp, scale=-0.5)
                nc.sync.dma_start(rsd[r0:r0 + 2, off:off + cw],
                                  pkrs[:, 0:mcols])

            # rsd row base: q01->0, k01->2, q23->4, k23->6
            def norm_part(p, which, off, cw, rsb):
                if which == "q":
                    dst, r0, src_m = qhat[p], 4 * p, p
                else:
                    dst, r0, src_m = khat[p], 4 * p + 2, 2 + p
                row = rsd[r0][off:off + cw]
                src = bass.AP(tensor=row.tensor, offset=row.offset,
                              ap=[[N, 2], [0, 64]] + list(row.ap))
                nc.sync.dma_start(rsb[:, off:off + cw], src)
                with nc.allow_low_precision(reason="bf16 norm mul"):
                    nc.vector.tensor_mul(dst[:, off:off + cw],
                                         qk_sb[src_m][:, off:off + cw],
                                         rsb[:, off:off + cw])

            def vt_tile(jt, pool=None, act_copy=False):
                pv = (pool or PSF).tile([128, 256], F32,
                                        tag="pf" if pool is None else "ps",
                                        name="pv")
                for c in range(2):
                    nc.tensor.matmul(
                        pv[:, :],
                        mc(x_sb[c][:, jt * 128:(jt + 1) * 128]),
                        mc(wvT_sb[c][:, :]),
                        start=(c == 0), stop=(c == 1))
                if act_copy:
                    nc.scalar.activation(
                        vT_sb[:, jt, :, :],
                        pv.rearrange("p (h d) -> p h d", h=4), ActCopy)
                else:
                    nc.vector.tensor_copy(
                        vT_sb[:, jt, :, :],
                        pv.rearrange("p (h d) -> p h d", h=4))

            def recip_chunk(hp, off, cw):
                """packed 1/s for pair hp chunk: s8 rows {64hp, 64hp+32}
                -> [128, m] -> DVE reciprocal -> rsdd[2hp:2hp+2].
                (reciprocal costs ~7 cyc/free-elem, so packing across
                partitions is essential)"""
                mcols = (2 * cw) // 128
                pks = PK.tile([128, 8], F32, tag="pks", name="pks")
                pkr = PK.tile([128, 8], WD, tag="pko", name="pkr")
                nc.sync.dma_start(pks[0:64, 0:mcols],
                                  s8[64 * hp:64 * hp + 1, off:off + cw])
                nc.sync.dma_start(pks[64:128, 0:mcols],
                                  s8[64 * hp + 32:64 * hp + 33,
                                     off:off + cw])
                with nc.allow_low_precision(reason="bf16 1/s"):
                    nc.vector.reciprocal(pkr[:, 0:mcols], pks[:, 0:mcols])
                nc.sync.dma_start(rsdd[2 * hp:2 * hp + 2, off:off + cw],
                                  pkr[:, 0:mcols])

            def nsc_chunk(hp, off, cw, rsb):
                row = rsdd[2 * hp][off:off + cw]
                src = bass.AP(tensor=row.tensor, offset=row.offset,
                              ap=[[N, 2], [0, 64]] + list(row.ap))
                nc.sync.dma_start(rsb[:, off:off + cw], src)
                with nc.allow_low_precision(reason="bf16 nsc mul"):
                    nc.vector.tensor_mul(nsc[hp][:, off:off + cw],
                                         numer[hp][:, off:off + cw],
                                         rsb[:, off:off + cw])

            def tail_scale(hp, off, cw):
                """latency-optimized last-chunk 1/s: row reciprocal (no DMA
                bounce) + K=1 ones-matmul partition broadcast."""
                b = 64 * hp
                with nc.allow_low_precision(reason="bf16 1/s"):
                    nc.vector.reciprocal(srec[b:b + 33, off:off + cw],
                                         s8[b:b + 33, off:off + cw])
                pb = PSF.tile([128, 512], F32, tag="pf", name="pb")
                nc.tensor.matmul(
                    pb[0:64, 0:cw],
                    mc(ones8_sb[b:b + 1, 9:73]),
                    mc(srec[b:b + 1, off:off + cw]),
                    start=True, stop=True, tile_position=(b, 0))
                nc.tensor.matmul(
                    pb[64:128, 0:cw],
                    mc(ones8_sb[b + 32:b + 33, 9:73]),
                    mc(srec[b + 32:b + 33, off:off + cw]),
                    start=True, stop=True, tile_position=(b + 32, 64))
                with nc.allow_low_precision(reason="bf16 nsc mul"):
                    nc.vector.tensor_mul(nsc[hp][:, off:off + cw],
                                         numer[hp][:, off:off + cw],
                                         pb[:, 0:cw])

            def outproj_piece(pr, m2, off, cw):
                pf = PSF.tile([128, 512], F32, tag="pf", name="pf")
                nc.tensor.matmul(
                    pf[:, 0:cw],
                    mc(woT_sb[pr][:, m2 * 128:(m2 + 1) * 128]),
                    mc(nsc[pr][:, off:off + cw]),
                    start=True, stop=True)
                yt = YST.tile([128, 512], F32, tag="yt", name="yt")
                if pr == 0:
                    nc.vector.tensor_scalar_add(yt[:, 0:cw], pf[:, 0:cw],
                                                bias_sb[m2][:, :])
                else:
                    nc.vector.tensor_copy(yt[:, 0:cw], pf[:, 0:cw])
                nc.sync.dma_start(y[pr][m2][:, off:off + cw], yt[:, 0:cw])

            # ---- attention core (ACT-bound steady state) ----
            # side: deque of thunks issued between j-groups (PE slack);
            # cadence: issue one thunk every `cadence` groups.
            def attention_pair(hp, off, cw, side=None, cadence=3):
                po = PSO.tile([128, 512], F32, tag="po", name="po")
                po_o = PSO.tile([33, 512], F32, tag="po_o", name="po_o")

                def sim_pair(jt, ps):
                    js = slice(jt * 128, (jt + 1) * 128)
                    nc.tensor.matmul(
                        ps[:, 0:cw],
                        mc(khat[hp][0:64, js]),
                        mc(qhat[hp][0:64, off:off + cw]),
                        start=True, stop=True, tile_position=(0, 0))
                    nc.tensor.matmul(
                        ps[:, 512:512 + cw],
                        mc(khat[hp][64:128, js]),
                        mc(qhat[hp][64:128, off:off + cw]),
                        start=True, stop=True, tile_position=(64, 0))

                def ev_group(jt, eh0, eh1):
                    st, sp = (jt == 0), (jt == NJ - 1)
                    nc.tensor.matmul(
                        po[0:64, 0:cw],
                        mc(vT_sb[:, jt, 2 * hp, :]),
                        mc(eh0),
                        start=st, stop=sp, tile_position=(0, 0),
                        skip_group_check=True)
                    nc.tensor.matmul(
                        po[64:128, 0:cw],
                        mc(vT_sb[:, jt, 2 * hp + 1, :]),
                        mc(eh1),
                        start=st, stop=sp, tile_position=(0, 64),
                        skip_group_check=True)
                    nc.tensor.matmul(
                        po_o[0:1, 0:cw],
                        mc(ones8_sb[:, 8:9]),
                        mc(eh0),
                        start=st, stop=sp, tile_position=(0, 0),
                        skip_group_check=True)
                    nc.tensor.matmul(
                        po_o[32:33, 0:cw],
                        mc(ones8_sb[:, 8:9]),
                        mc(eh1),
                        start=st, stop=sp, tile_position=(0, 32),
                        skip_group_check=True)

                # E@v trails 3 j's behind so the next chunk's first E@v
                # (which waits the previous chunk's po drain) never blocks
                # early sims on the in-order PE
                pend = []
                for jt in range(NJ):
                    ps = PSSIM.tile([128, 1024], F32, tag="ps", name="ps")
                    sim_pair(jt, ps)
                    e = ESB.tile([128, 1024], WD, tag="e", name="e")
                    ps3 = ps.rearrange("p (b c) -> p b c", b=2)
                    e3b = e.rearrange("p (b c) -> p b c", b=2)
                    nc.scalar.activation(e3b[:, :, 0:cw],
                                         ps3[:, :, 0:cw], Exp)
                    pend.append((jt, e))
                    if len(pend) > 3:
                        j0, ee = pend.pop(0)
                        ev_group(j0, ee[:, 0:cw], ee[:, 512:512 + cw])
                    if side and (jt % cadence == cadence - 1):
                        if len(side):
                            side.popleft()()
                for (j0, ee) in pend:
                    ev_group(j0, ee[:, 0:cw], ee[:, 512:512 + cw])
                # drain numerators + denominators (s rows at base 32h)
                nc.vector.tensor_copy(numer[hp][:, off:off + cw],
                                      po[:, 0:cw])
                for t in range(2):
                    h = 2 * hp + t
                    nc.vector.tensor_copy(
                        s8[32 * h:32 * h + 1, off:off + cw],
                        po_o[32 * t:32 * t + 1, 0:cw])

            # ---- schedule ----
            # PE warm-up: ~4us of dummy matmuls during the input-DMA wait
            # flips the HAM clock gate to 2.4GHz before real work arrives.
            wtile = SQ.tile([128, 512], WD, tag="q2", name="wtile")
            nc.vector.memset(wtile[:, :], 0.0)
            for _ in range(10):
                pw = PSF.tile([128, 512], F32, tag="pf", name="pw")
                nc.tensor.matmul(pw[:, :], mc(wtile[:, 0:128]),
                                 mc(wtile[:, :]), start=True, stop=True)

            # ramp: k01 full row (chunk-pipelined rs), q01 chunk 0, v^T via
            # the idle PSSIM banks; sim j-tiles span all of khat so the k
            # row must be complete, qhat only needs chunk 0.
            rsbq0 = RSB.tile([128, N], WD, tag="rsbq0", name="rsbq0")
            rsbk0 = RSB.tile([128, N], WD, tag="rsbk0", name="rsbk0")
            rsbq1 = RSB.tile([128, N], WD, tag="rsbq1", name="rsbq1")
            rsbk1 = RSB.tile([128, N], WD, tag="rsbk1", name="rsbk1")

            off0, cw0 = CHUNKS[0]
            for (off, cw) in CHUNKS:
                qkv_chunk(2, off, cw, act_copy=True)    # k01 (ACT is idle)
            rs_pack(32, 2, 0, N, qscale=False)
            qkv_chunk(0, off0, cw0, act_copy=True)      # q01 chunk 0
            rs_pack(0, 0, off0, cw0, qscale=True)
            for jt in range(6):
                vt_tile(jt, pool=PSSIM)
            norm_part(0, "k", 0, N, rsbk0)
            norm_part(0, "q", off0, cw0, rsbq0)

            # side work queues; prep for pair-0 chunk c (qhat) must finish
            # inside chunks 0..c-1; pair-1's k row inside pair-0's window.
            side_lists = [deque() for _ in CHUNKS]

            def make_qkv_thunk(m, off, cw):
                return lambda: qkv_chunk(m, off, cw)

            def make_rs_thunk(base, r0, off, cw, qs):
                return lambda: rs_pack(base, r0, off, cw, qs)

            def make_norm_thunk(p, which, off, cw, rsb):
                return lambda: norm_part(p, which, off, cw, rsb)

            def add_q_prep(dst, p, ci):
                off, cw = CHUNKS[ci]
                m_q = 0 if p == 0 else 1
                rq = rsbq0 if p == 0 else rsbq1
                dst.append(make_qkv_thunk(m_q, off, cw))
                dst.append(make_rs_thunk(64 * p, 4 * p, off, cw, True))
                dst.append(make_norm_thunk(p, "q", off, cw, rq))

            def make_vt_thunk(jt):
                return lambda: vt_tile(jt)

            for jt in range(6, NJ):
                side_lists[0].append(make_vt_thunk(jt))
            add_q_prep(side_lists[0], 0, 1)
            add_q_prep(side_lists[1], 0, 2)
            add_q_prep(side_lists[1], 0, 3)
            add_q_prep(side_lists[2], 0, 4)
            # pair-1 k row: project k23 + one full-row rs + chunked norm
            for (off, cw) in CHUNKS:
                side_lists[2].append(make_qkv_thunk(3, off, cw))
            side_lists[3].append(make_rs_thunk(96, 6, 0, N, False))
            for (off, cw) in CHUNKS:
                side_lists[3].append(
                    make_norm_thunk(1, "k", off, cw, rsbk1))
            add_q_prep(side_lists[3], 1, 0)

            cadences0 = [1, 3, 2, 2, 2]
            for ci, (off, cw) in enumerate(CHUNKS):
                attention_pair(0, off, cw, side=side_lists[ci],
                               cadence=cadences0[ci])
                while side_lists[ci]:
                    side_lists[ci].popleft()()

            # pair-1 attention with pair-0 scaling/outproj, remaining pair-1
            # q prep, and trailing pair-1 scaling/outproj as side work
            side1 = [deque() for _ in CHUNKS]

            def make_recip_thunk(hp, off, cw):
                return lambda: recip_chunk(hp, off, cw)

            rsb0 = RSB.tile([128, N], WD, tag="rsb0", name="rsb0")
            rsb1 = RSB.tile([128, N], WD, tag="rsb1", name="rsb1")

            def make_nsc_thunk(hp, off, cw):
                rsb = rsb0 if hp == 0 else rsb1
                return lambda: nsc_chunk(hp, off, cw, rsb)

            def make_out_thunk(pr, m2, off, cw):
                return lambda: outproj_piece(pr, m2, off, cw)

            for ci in range(1, len(CHUNKS)):
                add_q_prep(side1[ci - 1], 1, ci)
            for ci, (off, cw) in enumerate(CHUNKS):
                side1[ci].append(make_recip_thunk(0, off, cw))
                side1[ci].append(make_nsc_thunk(0, off, cw))
                side1[ci].append(make_out_thunk(0, 0, off, cw))
                side1[ci].append(make_out_thunk(0, 1, off, cw))
                if ci >= 1:
                    poff, pcw = CHUNKS[ci - 1]
                    side1[ci].append(make_recip_thunk(1, poff, pcw))
                    side1[ci].append(make_nsc_thunk(1, poff, pcw))
                    side1[ci].append(make_out_thunk(1, 0, poff, pcw))
                    side1[ci].append(make_out_thunk(1, 1, poff, pcw))

            cadences1 = [2, 1, 1, 1, 2]
            for ci, (off, cw) in enumerate(CHUNKS):
                attention_pair(1, off, cw, side=side1[ci],
                               cadence=cadences1[ci])
                while side1[ci]:
                    side1[ci].popleft()()

            # tail: last chunk of pair-1 scaling + outproj (low-latency
            # path: no DRAM bounce)
            offl, cwl = CHUNKS[-1]
            tail_scale(1, offl, cwl)
            outproj_piece(1, 0, offl, cwl)
            outproj_piece(1, 1, offl, cwl)

    nc.compile()
    return nc


def _get_program(wd_name=WD_NAME):
    if wd_name not in _CACHE:
        _CACHE[wd_name] = _build(wd_name)
    return _CACHE[wd_name]


def _np_wd(wd_name):
    if wd_name == "bf16":
        import ml_dtypes
        return np.dtype(ml_dtypes.bfloat16)
    return np.dtype(np.float32)


def make_in_maps(x, w_qkv, w_out, b_out, wd_name=WD_NAME):
    x = np.asarray(x, np.float32)
    w_qkv = np.asarray(w_qkv, np.float32)
    w_out = np.asarray(w_out, np.float32)
    b_out = np.asarray(b_out, np.float32)
    wd = _np_wd(wd_name)

    ones8 = np.zeros((128, 73), np.float32)
    ones8[:, 8:] = 1.0
    for cc in range(8):
        lo = 64 * (cc % 2)
        ones8[lo:lo + 64, cc] = 1.0

    in_maps = []
    for core in range(8):
        b, half = core // 2, core % 2
        hsel = slice(256 * half, 256 * (half + 1))
        q_rows = np.arange(0, 512)[hsel]
        k_rows = 512 + q_rows
        v_rows = 1024 + q_rows
        wqk_h = np.ascontiguousarray(
            w_qkv[np.r_[q_rows, k_rows], :].T).reshape(2, 128, 512)
        wvT_h = np.ascontiguousarray(w_qkv[v_rows, :].T).reshape(2, 128, 256)
        woT_h = np.ascontiguousarray(w_out[:, hsel].T).reshape(2, 128, 256)
        bias_h = (b_out if half == 0 else np.zeros_like(b_out))
        in_maps.append({
            "x2": x[b].reshape(C, N).reshape(2, 128, N).astype(wd),
            "wqk": wqk_h.astype(wd),
            "wvT": wvT_h.astype(wd),
            "woT": woT_h.astype(wd),
            "bias": bias_h.reshape(2, 128, 1).astype(np.float32),
            "ones8": ones8.astype(wd),
        })
    return in_maps


def gather_output(results):
    outs = [r["y"].sum(axis=0).reshape(C, N) for r in results]
    return np.stack([
        (outs[2 * b] + outs[2 * b + 1]).reshape(C, H, W) for b in range(B)
    ]).astype(np.float32)


def run(in_maps, wd_name=WD_NAME, **kwargs):
    from concourse import bass_utils
    nc = _get_program(wd_name)
    return bass_utils.run_bass_kernel_spmd(nc, in_maps,
                                           core_ids=list(range(8)), **kwargs)


def kernel(x, w_qkv, w_out, b_out):
    in_maps = make_in_maps(x, w_qkv, w_out, b_out)
    res = run(in_maps)
    return gather_output(res.results)


# revision 21
# speedup vs baseline: 1.2423x; 1.0971x over previous
"""Trainium2 Bass kernel for nn_Attention_21895743275585.

Reference computation (per batch b of 4):
  qkv = w_qkv @ x_flat            # 1x1 conv, x_flat [C=256, N=2304]
  q,k l2-normalized per (head, n) along dim_head=64; SCALE=10
  sim = 10 * qhat^T khat per head; attn = softmax(sim, axis=-1)
  out = attn @ v; final = w_out @ out_inner + b_out

Sharding: 8 cores = (batch b, head-half). Each core handles 4 of the 8 heads
of one batch; host sums the partial output projections (2 halves x 2 head
pairs per batch; bias is fed only to half 0 / pair 0).

The kernel is ACT(scalar engine)-bound: exp of the [N, N] sim matrices is
4 heads x 2304^2 = 21.2M elements at 1 elem/cycle/lane @1.2GHz ~= 160us.
Everything else is scheduled to hide under the exp stream:
  - attention runs chunk-major; each (chunk, j) group is one [128, 1024]
    PSUM->SBUF exp ACTIVATE (996ns steady cadence).
  - all other work (remaining QKV projection, v^T, norms, output projection,
    1/s scaling) is issued as small "side thunks" between attention groups so
    the PE/DVE/DMA work lands in the ACT-bound slack.
  - per-row math (1/sqrt(ss), 1/s) is repacked into dense [128, m] tiles via
    SBUF->SBUF/DRAM reshape DMAs so ACT/DVE process 128 partitions instead
    of 2-4 rows ([1,N]->[64,N] partition broadcasts bounce through small
    DRAM tensors -- DRAM APs allow a step-0 partition dim).
  - sim^T chunk [j, i] = k^T q in PSUM (two heads row-packed via
    tile_position); E@v packs two heads column-wise; denominators come from
    ones-column matmuls; exp needs no max subtraction since |sim|<=10.
  - Ln and Exp share one ACT table set (pinned natural_log_exp_and_others);
    rsqrt is exp(-0.5 ln x); 1/s is DVE reciprocal on packed tiles.
"""

import math
from collections import deque

import numpy as np

B, C, H, W = 4, 256, 48, 48
HEADS, DIM_HEAD, SCALE = 8, 64, 10.0
INNER = HEADS * DIM_HEAD
N = H * W                      # 2304
NJ = N // 128                  # 18 j-tiles
CHUNKS = [(0, 512), (512, 512), (1024, 512), (1536, 512), (2048, 256)]
EPS = 1e-12

WD_NAME = "bf16"               # working dtype: "bf16" | "f32r" | "f32"

_CACHE = {}


def _pin_act_tables():
    """Force every activation onto the natural_log_exp_and_others set so the
    whole kernel needs exactly one ACT table load (Ln+Exp share that set)."""
    import concourse.bacc as bacc_mod
    if getattr(bacc_mod, "_act_tables_pinned", False):
        return
    orig = bacc_mod.get_activation_tables

    def patched(arch):
        t = orig(arch)
        keep = "natural_log_exp_and_others"
        if keep in t:
            return {name: (funcs if name == keep else set())
                    for name, funcs in t.items()}
        return t

    bacc_mod.get_activation_tables = patched
    bacc_mod._act_tables_pinned = True


def _build(wd_name):
    import concourse.bass as bass
    import concourse.tile as tile
    from concourse import bacc, mybir

    _pin_act_tables()

    F32 = mybir.dt.float32
    F32R = mybir.dt.float32r
    WD = mybir.dt.bfloat16 if wd_name == "bf16" else F32

    def mc(ap):
        # matmul operand cast for the fast-fp32 PE path
        return ap.bitcast(F32R) if wd_name == "f32r" else ap

    Ln = mybir.ActivationFunctionType.Ln
    Exp = mybir.ActivationFunctionType.Exp
    ActCopy = mybir.ActivationFunctionType.Copy

    nc = bacc.Bacc("TRN2", target_bir_lowering=False, debug=False,
                   enable_asserts=False, num_devices=8)
    x2 = nc.dram_tensor("x2", [2, 128, N], WD, kind="ExternalInput").ap()
    wqk = nc.dram_tensor("wqk", [2, 128, 512], WD, kind="ExternalInput").ap()
    wvT = nc.dram_tensor("wvT", [2, 128, 256], WD, kind="ExternalInput").ap()
    woT = nc.dram_tensor("woT", [2, 128, 256], WD, kind="ExternalInput").ap()
    bias = nc.dram_tensor("bias", [2, 128, 1], F32, kind="ExternalInput").ap()
    ones8 = nc.dram_tensor("ones8", [128, 73], WD, kind="ExternalInput").ap()
    # output: per head-pair partial projections, summed on host
    y = nc.dram_tensor("y", [2, 2, 128, N], F32, kind="ExternalOutput").ap()
    # internal DRAM bounce rows for partition broadcasts (bf16: halves DMA
    # bytes and enables 2x DVE muls; ~0.2% rounding is within budget)
    # rsd rows: [q01(2), k01(2), q23(2), k23(2)] = 1/sqrt(ss) (q rows x10)
    rsd = nc.dram_tensor("rsd", [8, N], WD, kind="Internal").ap()
    rsdd = nc.dram_tensor("rsdd", [4, N], WD, kind="Internal").ap()

    with tile.TileContext(nc) as tc:
        with tc.tile_pool(name="persist", bufs=1) as P, \
             tc.tile_pool(name="bcast", bufs=1) as RSB, \
             tc.tile_pool(name="sq", bufs=3) as SQ, \
             tc.tile_pool(name="pk", bufs=2) as PK, \
             tc.tile_pool(name="esb", bufs=12) as ESB, \
             tc.tile_pool(name="yst", bufs=3) as YST, \
             tc.tile_pool(name="psf", bufs=2, space="PSUM") as PSF, \
             tc.tile_pool(name="pssim", bufs=2, space="PSUM") as PSSIM, \
             tc.tile_pool(name="pso", bufs=1, space="PSUM") as PSO:

            # ---- load inputs (wqk + x first: they gate the ramp) ----
            x_sb = [P.tile([128, N], WD, tag=f"x{c}", name=f"x{c}")
                    for c in range(2)]
            wqk_sb = [P.tile([128, 512], WD, tag=f"wqk{c}", name=f"wqk{c}")
                      for c in range(2)]
            wvT_sb = [P.tile([128, 256], WD, tag=f"wvT{c}", name=f"wvT{c}")
                      for c in range(2)]
            woT_sb = [P.tile([128, 256], WD, tag=f"woT{c}", name=f"woT{c}")
                      for c in range(2)]
            bias_sb = [P.tile([128, 1], F32, tag=f"bias{c}", name=f"bias{c}")
                       for c in range(2)]
            ones8_sb = P.tile([128, 73], WD, tag="ones8", name="ones8")
            for c in range(2):
                nc.sync.dma_start(wqk_sb[c][:, :], wqk[c])
                nc.sync.dma_start(x_sb[c][:, :], x2[c])
            nc.scalar.dma_start(ones8_sb[:, :], ones8)
            for c in range(2):
                nc.scalar.dma_start(wvT_sb[c][:, :], wvT[c])
                nc.scalar.dma_start(woT_sb[c][:, :], woT[c])
                nc.scalar.dma_start(bias_sb[c][:, :], bias[c])

            # all-partitions exp bias ln(SCALE) for q-row rsqrt packs
            biasq = P.tile([128, 1], F32, tag="biasq", name="biasq")
            nc.vector.memset(biasq[:, :], math.log(SCALE))

            qk_sb = [P.tile([128, N], WD, tag=f"qk{m}", name=f"qk{m}")
                     for m in range(4)]
            ss8 = P.tile([128, N], F32, tag="ss8", name="ss8")
            qhat = [P.tile([128, N], WD, tag=f"qh{p}", name=f"qh{p}")
                    for p in range(2)]
            khat = [P.tile([128, N], WD, tag=f"kh{p}", name=f"kh{p}")
                    for p in range(2)]
            vT_sb = P.tile([128, NJ, 4, 64], WD, tag="vT", name="vT")

            numer = [P.tile([128, N], WD, tag=f"nu{p}", name=f"nu{p}")
                     for p in range(2)]
            nsc = [P.tile([128, N], WD, tag=f"nsc{p}", name=f"nsc{p}")
                   for p in range(2)]
            s8 = P.tile([128, N], F32, tag="s8", name="s8")
            srec = P.tile([128, N], WD, tag="srec", name="srec")
            nc.vector.memset(s8[:, :], 1.0)

            # ---- building blocks ----

            # m tile channel order: 0=q01, 1=q23, 2=k01, 3=k23
            # ss8 row base for m: q01->0, k01->32, q23->64, k23->96
            SS_BASE = {0: 0, 2: 32, 1: 64, 3: 96}

            def qkv_chunk(m, off, cw, act_copy=False):
                """project chunk [off, off+cw) of qk tile m + its sum-sq."""
                base = SS_BASE[m]
                pq = PSF.tile([128, 512], F32, tag="pf", name="pq")
                for c in range(2):
                    nc.tensor.matmul(
                        pq[:, 0:cw],
                        mc(wqk_sb[c][:, m * 128:(m + 1) * 128]),
                        mc(x_sb[c][:, off:off + cw]),
                        start=(c == 0), stop=(c == 1))
                if act_copy:
                    nc.scalar.activation(qk_sb[m][:, off:off + cw],
                                         pq[:, 0:cw], ActCopy)
                else:
                    nc.vector.tensor_copy(qk_sb[m][:, off:off + cw],
                                          pq[:, 0:cw])
                q2 = SQ.tile([128, 512], WD, tag="q2", name="q2")
                nc.vector.tensor_mul(q2[:, 0:cw],
                                     qk_sb[m][:, off:off + cw],
                                     qk_sb[m][:, off:off + cw])
                pss = PSF.tile([8, 512], F32, tag="pf", name="pss")
                nc.tensor.matmul(pss[:, 0:cw], mc(ones8_sb[:, 0:8]),
                                 mc(q2[:, 0:cw]), start=True, stop=True)
                if act_copy:
                    nc.scalar.activation(ss8[base:base + 2, off:off + cw],
                                         pss[0:2, 0:cw], ActCopy)
                else:
                    nc.vector.tensor_copy(ss8[base:base + 2, off:off + cw],
                                          pss[0:2, 0:cw])

            def rs_pack(ss_base, r0, off, cw, qscale):
                """packed 1/sqrt(ss): two ss8 rows [2, off:off+cw] ->
                [128, m] -> clamp, ln, exp(-0.5 ln + bias) -> rsd[r0:r0+2].
                q rows get bias=ln(SCALE) (rs = SCALE/||q||)."""
                mcols = (2 * cw) // 128
                pkin = PK.tile([128, 36], F32, tag="pk", name="pkin")
                pkrs = PK.tile([128, 36], WD, tag="pko", name="pkrs")
                nc.sync.dma_start(
                    pkin[:, 0:mcols],
                    ss8[ss_base:ss_base + 2, off:off + cw])
                nc.vector.tensor_scalar_max(pkin[:, 0:mcols],
                                            pkin[:, 0:mcols], EPS * EPS)
                nc.scalar.activation(pkin[:, 0:mcols], pkin[:, 0:mcols], Ln)
                if qscale:
                    nc.scalar.activation(pkrs[:, 0:mcols], pkin[:, 0:mcols],
                                         Exp, scale=-0.5, bias=biasq[:, :])
                else:
                    nc.scalar.activation(pkrs[:, 0:mcols], pkin[:, 0:mcols],
                                         Exp, scale=-0.5)
                nc.sync.dma_start(rsd[r0:r0 + 2, off:off + cw],
                                  pkrs[:, 0:mcols])

            # rsd row base: q01->0, k01->2, q23->4, k23->6
            def norm_part(p, which, off, cw, rsb):
                if which == "q":
                    dst, r0, src_m = qhat[p], 4 * p, p
                else:
                    dst, r0, src_m = khat[p], 4 * p + 2, 2 + p
                for t in range(2):
                    row = rsd[r0 + t][off:off + cw]
                    src = bass.AP(tensor=row.tensor, offset=row.offset,
                                  ap=[[0, 64]] + list(row.ap))
                    nc.sync.dma_start(rsb[64 * t:64 * t + 64, off:off + cw],
                                      src)
                with nc.allow_low_precision(reason="bf16 norm mul"):
                    nc.vector.tensor_mul(dst[:, off:off + cw],
                                         qk_sb[src_m][:, off:off + cw],
                                         rsb[:, off:off + cw])

            def vt_tile(jt, pool=None, act_copy=False):
                pv = (pool or PSF).tile([128, 256], F32,
                                        tag="pf" if pool is None else "ps",
                                        name="pv")
                for c in range(2):
                    nc.tensor.matmul(
                        pv[:, :],
                        mc(x_sb[c][:, jt * 128:(jt + 1) * 128]),
                        mc(wvT_sb[c][:, :]),
                        start=(c == 0), stop=(c == 1))
                if act_copy:
                    nc.scalar.activation(
                        vT_sb[:, jt, :, :],
                        pv.rearrange("p (h d) -> p h d", h=4), ActCopy)
                else:
                    nc.vector.tensor_copy(
                        vT_sb[:, jt, :, :],
                        pv.rearrange("p (h d) -> p h d", h=4))

            def recip_chunk(hp, off, cw):
                """packed 1/s for pair hp chunk: s8 rows {64hp, 64hp+32}
                -> [128, m] -> DVE reciprocal -> rsdd[2hp:2hp+2].
                (reciprocal costs ~7 cyc/free-elem, so packing across
                partitions is essential)"""
                mcols = (2 * cw) // 128
                pks = PK.tile([128, 8], F32, tag="pks", name="pks")
                pkr = PK.tile([128, 8], WD, tag="pko", name="pkr")
                nc.sync.dma_start(pks[0:64, 0:mcols],
                                  s8[64 * hp:64 * hp + 1, off:off + cw])
                nc.sync.dma_start(pks[64:128, 0:mcols],
                                  s8[64 * hp + 32:64 * hp + 33,
                                     off:off + cw])
                with nc.allow_low_precision(reason="bf16 1/s"):
                    nc.vector.reciprocal(pkr[:, 0:mcols], pks[:, 0:mcols])
                nc.sync.dma_start(rsdd[2 * hp:2 * hp + 2, off:off + cw],
                                  pkr[:, 0:mcols])

            def nsc_chunk(hp, off, cw, rsb):
                for t in range(2):
                    row = rsdd[2 * hp + t][off:off + cw]
                    src = bass.AP(tensor=row.tensor, offset=row.offset,
                                  ap=[[0, 64]] + list(row.ap))
                    nc.sync.dma_start(rsb[64 * t:64 * t + 64, off:off + cw],
                                      src)
                with nc.allow_low_precision(reason="bf16 nsc mul"):
                    nc.vector.tensor_mul(nsc[hp][:, off:off + cw],
                                         numer[hp][:, off:off + cw],
                                         rsb[:, off:off + cw])

            def tail_scale(hp, off, cw):
                """latency-optimized last-chunk 1/s: row reciprocal (no DMA
                bounce) + K=1 ones-matmul partition broadcast."""
                b = 64 * hp
                with nc.allow_low_precision(reason="bf16 1/s"):
                    nc.vector.reciprocal(srec[b:b + 33, off:off + cw],
                                         s8[b:b + 33, off:off + cw])
                pb = PSF.tile([128, 512], F32, tag="pf", name="pb")
                nc.tensor.matmul(
                    pb[0:64, 0:cw],
                    mc(ones8_sb[b:b + 1, 9:73]),
                    mc(srec[b:b + 1, off:off + cw]),
                    start=True, stop=True, tile_position=(b, 0))
                nc.tensor.matmul(
                    pb[64:128, 0:cw],
                    mc(ones8_sb[b + 32:b + 33, 9:73]),
                    mc(srec[b + 32:b + 33, off:off + cw]),
                    start=True, stop=True, tile_position=(b + 32, 64))
                with nc.allow_low_precision(reason="bf16 nsc mul"):
                    nc.vector.tensor_mul(nsc[hp][:, off:off + cw],
                                         numer[hp][:, off:off + cw],
                                         pb[:, 0:cw])

            def outproj_piece(pr, m2, off, cw):
                pf = PSF.tile([128, 512], F32, tag="pf", name="pf")
                nc.tensor.matmul(
                    pf[:, 0:cw],
                    mc(woT_sb[pr][:, m2 * 128:(m2 + 1) * 128]),
                    mc(nsc[pr][:, off:off + cw]),
                    start=True, stop=True)
                yt = YST.tile([128, 512], F32, tag="yt", name="yt")
                if pr == 0:
                    nc.vector.tensor_scalar_add(yt[:, 0:cw], pf[:, 0:cw],
                                                bias_sb[m2][:, :])
                else:
                    nc.vector.tensor_copy(yt[:, 0:cw], pf[:, 0:cw])
                nc.sync.dma_start(y[pr][m2][:, off:off + cw], yt[:, 0:cw])

            # ---- attention core (ACT-bound steady state) ----
            # side: deque of thunks issued between j-groups (PE slack);
            # cadence: issue one thunk every `cadence` groups.
            def attention_pair(hp, off, cw, side=None, cadence=3):
                po = PSO.tile([128, 512], F32, tag="po", name="po")
                po_o = PSO.tile([33, 512], F32, tag="po_o", name="po_o")

                def sim_pair(jt, ps):
                    js = slice(jt * 128, (jt + 1) * 128)
                    nc.tensor.matmul(
                        ps[:, 0:cw],
                        mc(khat[hp][0:64, js]),
                        mc(qhat[hp][0:64, off:off + cw]),
                        start=True, stop=True, tile_position=(0, 0))
                    nc.tensor.matmul(
                        ps[:, 512:512 + cw],
                        mc(khat[hp][64:128, js]),
                        mc(qhat[hp][64:128, off:off + cw]),
                        start=True, stop=True, tile_position=(64, 0))

                def ev_group(jt, eh0, eh1):
                    st, sp = (jt == 0), (jt == NJ - 1)
                    nc.tensor.matmul(
                        po[0:64, 0:cw],
                        mc(vT_sb[:, jt, 2 * hp, :]),
                        mc(eh0),
                        start=st, stop=sp, tile_position=(0, 0),
                        skip_group_check=True)
                    nc.tensor.matmul(
                        po[64:128, 0:cw],
                        mc(vT_sb[:, jt, 2 * hp + 1, :]),
                        mc(eh1),
                        start=st, stop=sp, tile_position=(0, 64),
                        skip_group_check=True)
                    nc.tensor.matmul(
                        po_o[0:1, 0:cw],
                        mc(ones8_sb[:, 8:9]),
                        mc(eh0),
                        start=st, stop=sp, tile_position=(0, 0),
                        skip_group_check=True)
                    nc.tensor.matmul(
                        po_o[32:33, 0:cw],
                        mc(ones8_sb[:, 8:9]),
                        mc(eh1),
                        start=st, stop=sp, tile_position=(0, 32),
                        skip_group_check=True)

                # E@v trails 3 j's behind so the next chunk's first E@v
                # (which waits the previous chunk's po drain) never blocks
                # early sims on the in-order PE
                pend = []
                for jt in range(NJ):
                    ps = PSSIM.tile([128, 1024], F32, tag="ps", name="ps")
                    sim_pair(jt, ps)
                    e = ESB.tile([128, 1024], WD, tag="e", name="e")
                    ps3 = ps.rearrange("p (b c) -> p b c", b=2)
                    e3b = e.rearrange("p (b c) -> p b c", b=2)
                    nc.scalar.activation(e3b[:, :, 0:cw],
                                         ps3[:, :, 0:cw], Exp)
                    pend.append((jt, e))
                    if len(pend) > 3:
                        j0, ee = pend.pop(0)
                        ev_group(j0, ee[:, 0:cw], ee[:, 512:512 + cw])
                    if side and (jt % cadence == cadence - 1):
                        if len(side):
                            side.popleft()()
                for (j0, ee) in pend:
                    ev_group(j0, ee[:, 0:cw], ee[:, 512:512 + cw])
                # drain numerators + denominators (s rows at base 32h)
                nc.vector.tensor_copy(numer[hp][:, off:off + cw],
                                      po[:, 0:cw])
                for t in range(2):
                    h = 2 * hp + t
                    nc.vector.tensor_copy(
                        s8[32 * h:32 * h + 1, off:off + cw],
                        po_o[32 * t:32 * t + 1, 0:cw])

            # ---- schedule ----
            # PE warm-up: ~4us of dummy matmuls during the input-DMA wait
            # flips the HAM clock gate to 2.4GHz before real work arrives.
            wtile = SQ.tile([128, 512], WD, tag="q2", name="wtile")
            nc.vector.memset(wtile[:, :], 0.0)
            pw = PSF.tile([128, 512], F32, tag="pf", name="pw")
            for i in range(8):
                nc.tensor.matmul(pw[:, :], mc(wtile[:, 0:128]),
                                 mc(wtile[:, :]), start=(i == 0),
                                 stop=(i == 7))

            # ramp: k01 full row (chunk-pipelined rs), q01 chunk 0, v^T via
            # the idle PSSIM banks; sim j-tiles span all of khat so the k
            # row must be complete, qhat only needs chunk 0.
            rsbq0 = RSB.tile([128, N], WD, tag="rsbq0", name="rsbq0")
            rsbk0 = RSB.tile([128, N], WD, tag="rsbk0", name="rsbk0")
            rsbq1 = RSB.tile([128, N], WD, tag="rsbq1", name="rsbq1")
            rsbk1 = RSB.tile([128, N], WD, tag="rsbk1", name="rsbk1")

            off0, cw0 = CHUNKS[0]
            qkv_chunk(2, off0, cw0)                     # k01 chunk 0
            qkv_chunk(0, off0, cw0)                     # q01 chunk 0
            rs_pack(32, 2, off0, cw0, qscale=False)
            rs_pack(0, 0, off0, cw0, qscale=True)
            norm_part(0, "k", off0, cw0, rsbk0)
            norm_part(0, "q", off0, cw0, rsbq0)
            for ci in range(1, len(CHUNKS)):
                off, cw = CHUNKS[ci]
                qkv_chunk(2, off, cw)                   # k01 rest
                rs_pack(32, 2, off, cw, qscale=False)
                norm_part(0, "k", off, cw, rsbk0)
            for jt in range(6):
                vt_tile(jt, pool=PSSIM)

            # side work queues; prep for pair-0 chunk c (qhat) must finish
            # inside chunks 0..c-1; pair-1's k row inside pair-0's window.
            side_lists = [deque() for _ in CHUNKS]

            def make_qkv_thunk(m, off, cw):
                return lambda: qkv_chunk(m, off, cw)

            def make_rs_thunk(base, r0, off, cw, qs):
                return lambda: rs_pack(base, r0, off, cw, qs)

            def make_norm_thunk(p, which, off, cw, rsb):
                return lambda: norm_part(p, which, off, cw, rsb)

            def add_q_prep(dst, p, ci):
                off, cw = CHUNKS[ci]
                m_q = 0 if p == 0 else 1
                rq = rsbq0 if p == 0 else rsbq1
                dst.append(make_qkv_thunk(m_q, off, cw))
                dst.append(make_rs_thunk(64 * p, 4 * p, off, cw, True))
                dst.append(make_norm_thunk(p, "q", off, cw, rq))

            def make_vt_thunk(jt):
                return lambda: vt_tile(jt)

            for jt in range(6, NJ):
                side_lists[0].append(make_vt_thunk(jt))
            add_q_prep(side_lists[0], 0, 1)
            add_q_prep(side_lists[1], 0, 2)
            add_q_prep(side_lists[1], 0, 3)
            add_q_prep(side_lists[2], 0, 4)
            # pair-1 k row: project k23 + one full-row rs + chunked norm
            for (off, cw) in CHUNKS:
                side_lists[2].append(make_qkv_thunk(3, off, cw))
            side_lists[3].append(make_rs_thunk(96, 6, 0, N, False))
            for (off, cw) in CHUNKS:
                side_lists[3].append(
                    make_norm_thunk(1, "k", off, cw, rsbk1))
            add_q_prep(side_lists[3], 1, 0)

            cadences0 = [1, 3, 2, 2, 2]
            for ci, (off, cw) in enumerate(CHUNKS):
                attention_pair(0, off, cw, side=side_lists[ci],
                               cadence=cadences0[ci])
                while side_lists[ci]:
                    side_lists[ci].popleft()()

            # pair-1 attention with pair-0 scaling/outproj, remaining pair-1
            # q prep, and trailing pair-1 scaling/outproj as side work
            side1 = [deque() for _ in CHUNKS]

            def make_recip_thunk(hp, off, cw):
                return lambda: recip_chunk(hp, off, cw)

            rsb0 = RSB.tile([128, N], WD, tag="rsb0", name="rsb0")
            rsb1 = RSB.tile([128, N], WD, tag="rsb1", name="rsb1")

            def make_nsc_thunk(hp, off, cw):
                rsb = rsb0 if hp == 0 else rsb1
                return lambda: nsc_chunk(hp, off, cw, rsb)

            def make_out_thunk(pr, m2, off, cw):
                return lambda: outproj_piece(pr, m2, off, cw)

            # interleave prep between the scale-chain steps so each step's
            # multi-us DMA/DVE dependency is resolved before its PE consumer
            # enters the in-order PE queue
            for ci, (off, cw) in enumerate(CHUNKS):
                prep = deque()
                if ci + 1 < len(CHUNKS):
                    add_q_prep(prep, 1, ci + 1)
                sl = side1[ci]
                sl.append(make_recip_thunk(0, off, cw))
                if ci >= 1:
                    poff, pcw = CHUNKS[ci - 1]
                    sl.append(make_recip_thunk(1, poff, pcw))
                if prep:
                    sl.append(prep.popleft())
                if prep:
                    sl.append(prep.popleft())
                sl.append(make_nsc_thunk(0, off, cw))
                if ci >= 1:
                    sl.append(make_nsc_thunk(1, poff, pcw))
                if prep:
                    sl.append(prep.popleft())
                sl.append(make_out_thunk(0, 0, off, cw))
                if ci >= 1:
                    sl.append(make_out_thunk(1, 0, poff, pcw))
                sl.append(make_out_thunk(0, 1, off, cw))
                if ci >= 1:
                    sl.append(make_out_thunk(1, 1, poff, pcw))

            cadences1 = [2, 1, 1, 1, 2]
            for ci, (off, cw) in enumerate(CHUNKS):
                attention_pair(1, off, cw, side=side1[ci],
                               cadence=cadences1[ci])
                while side1[ci]:
                    side1[ci].popleft()()

            # tail: last chunk of pair-1 scaling + outproj (low-latency
            # path: no DRAM bounce)
            offl, cwl = CHUNKS[-1]
            tail_scale(1, offl, cwl)
            outproj_piece(1, 0, offl, cwl)
            outproj_piece(1, 1, offl, cwl)

    nc.compile()
    return nc


def _get_program(wd_name=WD_NAME):
    if wd_name not in _CACHE:
        _CACHE[wd_name] = _build(wd_name)
    return _CACHE[wd_name]


def _np_wd(wd_name):
    if wd_name == "bf16":
        import ml_dtypes
        return np.dtype(ml_dtypes.bfloat16)
    return np.dtype(np.float32)


def make_in_maps(x, w_qkv, w_out, b_out, wd_name=WD_NAME):
    x = np.asarray(x, np.float32)
    w_qkv = np.asarray(w_qkv, np.float32)
    w_out = np.asarray(w_out, np.float32)
    b_out = np.asarray(b_out, np.float32)
    wd = _np_wd(wd_name)

    ones8 = np.zeros((128, 73), np.float32)
    ones8[:, 8:] = 1.0
    for cc in range(8):
        lo = 64 * (cc % 2)
        ones8[lo:lo + 64, cc] = 1.0

    in_maps = []
    for core in range(8):
        b, half = core // 2, core % 2
        hsel = slice(256 * half, 256 * (half + 1))
        q_rows = np.arange(0, 512)[hsel]
        k_rows = 512 + q_rows
        v_rows = 1024 + q_rows
        wqk_h = np.ascontiguousarray(
            w_qkv[np.r_[q_rows, k_rows], :].T).reshape(2, 128, 512)
        wvT_h = np.ascontiguousarray(w_qkv[v_rows, :].T).reshape(2, 128, 256)
        woT_h = np.ascontiguousarray(w_out[:, hsel].T).reshape(2, 128, 256)
        bias_h = (b_out if half == 0 else np.zeros_like(b_out))
        in_maps.append({
            "x2": x[b].reshape(C, N).reshape(2, 128, N).astype(wd),
            "wqk": wqk_h.astype(wd),
            "wvT": wvT_h.astype(wd),
            "woT": woT_h.astype(wd),
            "bias": bias_h.reshape(2, 128, 1).astype(np.float32),
            "ones8": ones8.astype(wd),
        })
    return in_maps


def gather_output(results):
    outs = [r["y"].sum(axis=0).reshape(C, N) for r in results]
    return np.stack([
        (outs[2 * b] + outs[2 * b + 1]).reshape(C, H, W) for b in range(B)
    ]).astype(np.float32)


def run(in_maps, wd_name=WD_NAME, **kwargs):
    from concourse import bass_utils
    nc = _get_program(wd_name)
    return bass_utils.run_bass_kernel_spmd(nc, in_maps,
                                           core_ids=list(range(8)), **kwargs)


def kernel(x, w_qkv, w_out, b_out):
    in_maps = make_in_maps(x, w_qkv, w_out, b_out)
    res = run(in_maps)
    return gather_output(res.results)
